# revision 25
# baseline (speedup 1.0000x reference)
"""DPQ joint classification loss on 8 Trainium2 NeuronCores.

reference math (B=4096, D=512, C=10000):
    soft_pred = soft_x @ weight.T ; hard_pred = hard_x @ weight.T
    loss = CE(soft_pred, t) + CE(hard_pred, t)
           + 0.5 * 0.5*(||soft_x - centers[t]||^2 + ||hard_x - centers[t]||^2) / B

Sharding: data-parallel over batch. Core i gets soft rows [i*512,(i+1)*512)
and the matching hard rows, stacked into X = [1024, 512]; weight/centers are
replicated. Each core returns one scalar:
    sum_rows( logsumexp(X @ W^T) - logit_at_target + 0.25*||X - centers[t]||^2 )
and the host computes loss = sum(cores) / B.

Per-core pipeline:
  - PE: fp8e4m3 DoubleRow GEMM (2 fp8 MACs/cell/cycle), rows on partitions
    (8 chunks of 128), classes streamed in 512-wide PSUM banks (4 banks per
    group), fp32 accumulation over 2 double-k chunks of 256.  The weight is
    pre-scaled by 2^12 on the host so its +-0.024 entries land in e4m3's
    normal range; the 2^-12 is folded into the exp activation's free affine
    scale.  The logits only feed logsumexp, which is insensitive to the
    ~1-2% fp8 quantization noise; the target logit is computed exactly in
    fp32 on the DVE side path.
  - ACT: exp straight out of PSUM with fused per-row accumulation
    (no max-subtraction: logits are ~N(0, 0.31), exp is safe in fp32).
  - GPSIMD: indirect-DMA row gathers weight[targets], centers[targets].
  - DVE: exact fp32 target-logit (rowsum(x * w_gather)) and quantization
    (rowsum((x - c_gather)^2)) terms, final per-row combine.
  - PE again: cross-partition sum via ones-matmul; DMA scalar out.
"""

import json

import numpy as np

B_FULL = 4096
D = 512
C = 10000
N_CORES = 8
BS = B_FULL // N_CORES          # 512 rows per core per tensor
B = 2 * BS                      # 1024 stacked rows per core
P = 128
NB = B // P                     # 8 row chunks
NK = D // P                     # 4 contraction chunks
NKD = NK // 2                   # 2 DoubleRow chunks of 256
GW = 2048                       # class-group width = 4 PSUM banks
PARAM = 0.5
WSCALE = 4096.0                 # weight pre-scale: lifts fp8 w out of subnormals


def _patch_bir_bytes(b: bytes, max_waits: int = 1) -> bytes:
    """Adapt Tile-emitted BIR to this walrus build: it supports only one
    sync-wait per instruction (excess waits move to preceding NoOps) and
    rejects the EVENT_SEMAPHORE_RANGE_CLEAR raw-ISA encoding (replaced by
    per-semaphore write-0 EventSemaphore ops)."""
    d = json.loads(b)
    for f in d["functions"]:
        for blk in f["blocks"]:
            new_insts = []
            for ins in blk["instructions"]:
                if (
                    ins.get("opcode") == "ISA"
                    and ins.get("op_name") == "EVENT_SEMAPHORE_RANGE_CLEAR"
                ):
                    # distribute the clears round-robin over all engines:
                    # they sit between the two epilogue barriers, where every
                    # engine is idle, so 5-way parallel beats serial-on-Pool
                    ad = ins.get("ant_dict") or {}
                    engines = ["Pool", "Activation", "PE", "DVE", "SP"]
                    for j, sem_id in enumerate(
                        range(ad["range_first"], ad["range_last"] + 1)
                    ):
                        new_insts.append({
                            "name": f"{ins['name']}_clr{sem_id}",
                            "opcode": "EventSemaphore",
                            "engine": engines[j % len(engines)],
                            "ins": [],
                            "outs": [],
                            "debug": ins.get("debug"),
                            "sync_info": {
                                "on_wait": [],
                                "on_update": [{
                                    "ant_name": f"semclr_{sem_id}",
                                    "id": sem_id,
                                    "sync_type": "semaphore",
                                    "update_mode": "sem-wr-imm",
                                    "update_value": 0,
                                }],
                            },
                        })
                    continue
                si = ins.get("sync_info")
                waits = (si or {}).get("on_wait") or []
                if len(waits) > max_waits:
                    extra, keep = waits[:-max_waits], waits[-max_waits:]
                    idx = 0
                    while extra:
                        chunk, extra = extra[:max_waits], extra[max_waits:]
                        new_insts.append({
                            "name": f"{ins['name']}_w{idx}",
                            "opcode": "NoOp",
                            "engine": ins["engine"],
                            "ins": [],
                            "outs": [],
                            "debug": ins.get("debug"),
                            "sync_info": {"on_wait": chunk, "on_update": []},
                        })
                        idx += 1
                    si["on_wait"] = keep
                new_insts.append(ins)
            blk["instructions"] = new_insts
    return json.dumps(d).encode()


def _build_bass():
    import concourse.bass as bass
    import concourse.tile as tile
    from concourse import mybir

    f32 = mybir.dt.float32
    bf16 = mybir.dt.bfloat16
    fp8 = mybir.dt.float8e4
    i32 = mybir.dt.int32
    AF = mybir.ActivationFunctionType
    OP = mybir.AluOpType
    PM = mybir.MatmulPerfMode

    groups = []
    c0 = 0
    while c0 < C:
        groups.append((c0, min(GW, C - c0)))
        c0 += GW
    NG = len(groups)            # 5: 4 x 2048 + 1 x 1808

    nc = bass.Bass()
    # xt8[p, s, r] = X[r, s*128 + p]  (fp8e4m3)
    xt_d = nc.dram_tensor("xt", [P, NK, B], fp8, kind="ExternalInput")
    # aux path tensors in bf16: halves HBM traffic, precision is ample
    # (the aux terms only need ~1e-3 relative accuracy).
    x_d = nc.dram_tensor("x", [B, D], bf16, kind="ExternalInput")
    # wt8[p, s, c] = W[c, s*128 + p] * WSCALE  (fp8e4m3)
    wt_d = nc.dram_tensor("wt", [P, NK, C], fp8, kind="ExternalInput")
    w_d = nc.dram_tensor("w", [C, D], bf16, kind="ExternalInput")
    cen_d = nc.dram_tensor("cen", [C, D], bf16, kind="ExternalInput")
    tgt_d = nc.dram_tensor("tgt", [BS, 1], i32, kind="ExternalInput")
    out_d = nc.dram_tensor("out", [1, 1], f32, kind="ExternalOutput")

    with tile.TileContext(nc) as tc:
        with (
            tc.tile_pool(name="persist", bufs=1) as persist,
            tc.tile_pool(name="wtp", bufs=2) as wtp,
            tc.tile_pool(name="scratch", bufs=3) as scratch,
        ):
            # ---- resident loads ----
            # xt feeds the matmuls: load it first (split per double-k chunk)
            # so the PE can start as soon as the first half lands.
            xt_sb = persist.tile([P, NK, B], fp8, name="xt8")
            nc.sync.dma_start(xt_sb[:, 0:2, :], xt_d[:, 0:2, :])
            # x/tgt feed only the (small) DVE aux path; issue them on the
            # same sync HWDGE queue *behind* the first two weight groups so
            # they never compete with the GEMM-critical transfers (the aux
            # path has ~80us of slack).  gpsimd SWDGE handles the gathers.
            x_sb = []
            tgt_sb = []
            aux_loads = []

            def emit_aux_loads():
                for c in range(BS // P):
                    t = persist.tile([P, 1], i32, tag=f"tgt{c}", name=f"tgt{c}")
                    nc.sync.dma_start(t[:, :], tgt_d[c * P:(c + 1) * P, :])
                    tgt_sb.append(t)
                for b in range(NB):
                    t = persist.tile([P, D], bf16, tag=f"x{b}", name=f"x{b}")
                    nc.sync.dma_start(t[:, :], x_d[b * P:(b + 1) * P, :])
                    x_sb.append(t)

            # ---- gathers: weight[targets], centers[targets] ----
            wg_sb, cg_sb = [], []

            def emit_gathers():
                for c in range(BS // P):
                    wg = persist.tile([P, D], bf16, tag=f"wg{c}", name=f"wg{c}")
                    nc.gpsimd.indirect_dma_start(
                        out=wg[:, :], out_offset=None, in_=w_d[:, :],
                        in_offset=bass.IndirectOffsetOnAxis(
                            ap=tgt_sb[c][:, :1], axis=0),
                    )
                    wg_sb.append(wg)
                    cg = persist.tile([P, D], bf16, tag=f"cg{c}", name=f"cg{c}")
                    nc.gpsimd.indirect_dma_start(
                        out=cg[:, :], out_offset=None, in_=cen_d[:, :],
                        in_offset=bass.IndirectOffsetOnAxis(
                            ap=tgt_sb[c][:, :1], axis=0),
                    )
                    cg_sb.append(cg)

            # ---- small result tiles ----
            sums = persist.tile([P, NB * NG], f32, name="sums")
            se03 = persist.tile([P, NB], f32, name="se03")
            se = persist.tile([P, NB], f32, name="se")
            lse = persist.tile([P, NB], f32, name="lse")
            tcol = persist.tile([P, NB], f32, name="tcol")
            qcol = persist.tile([P, NB], f32, name="qcol")
            pre = persist.tile([P, NB], f32, name="pre")
            ctr2 = persist.tile([P, NB], f32, name="ctr2")
            rowtot = persist.tile([P, 1], f32, name="rowtot")
            ones = persist.tile([P, 1], f32, name="ones")
            nc.vector.memset(ones[:, :], 1.0)
            junk = persist.tile([P, 512], bf16, name="junk")
            nc.vector.memset(junk[:, :], 0.5)

            # ---- PE pre-warm ----
            # throwaway matmuls during the head DMA wait flip the HAM clock
            # gate to 8/8 (needs ~3.4us of sustained PE activity), so the
            # first real matmuls run at 2.4GHz instead of 1.2GHz.
            with tc.tile_pool(name="warm", bufs=1, space="PSUM") as warm_pool:
                wps = warm_pool.tile([P, 512], f32, name="wps")
                for i in range(7):
                    nc.tensor.matmul(
                        wps[:, :], lhsT=junk[:, :P], rhs=junk[:, :],
                        start=True, stop=True,
                    )

            # ---- aux path on DVE: target logits + quantization ----
            # (tensor_tensor_reduce lowers to a raw DVE ISA encoding this
            # walrus rejects, so use separate mul/sub + reduce ops; bf16
            # intermediates get the DVE 2x modes).  Emitted per row-block,
            # interleaved into the main loop so the DVE queue services them
            # between fast-exp tiles instead of serially at the end.
            def emit_aux_compute(b):
                c = b % (BS // P)
                pr = scratch.tile([P, D], bf16, tag="pr", name=f"pr{b}")
                nc.vector.tensor_mul(pr[:, :], x_sb[b][:, :], wg_sb[c][:, :])
                nc.vector.tensor_reduce(
                    out=tcol[:, b:b + 1], in_=pr[:, :],
                    axis=mybir.AxisListType.X, op=OP.add,
                )
                df = scratch.tile([P, D], bf16, tag="df", name=f"df{b}")
                nc.vector.tensor_sub(df[:, :], x_sb[b][:, :], cg_sb[c][:, :])
                sq = scratch.tile([P, D], bf16, tag="sq", name=f"sq{b}")
                nc.vector.tensor_mul(sq[:, :], df[:, :], df[:, :])
                nc.vector.tensor_reduce(
                    out=qcol[:, b:b + 1], in_=sq[:, :],
                    axis=mybir.AxisListType.X, op=OP.add,
                )

            # Fast exp on DVE for 2 of 8 row-blocks per group (ScalarE is
            # otherwise the bottleneck at ~82us vs PE ~78us).  Schraudolph:
            # bits(2^t) ~= A*t + B with the sawtooth mean-corrected via B;
            # psum is drained to an i32 SBUF tile first so the PSUM buffer
            # is freed at ACT-like pace, then reduced via a free bitcast.
            EXPA = (2.0 ** 23) * 1.4426950408889634 / WSCALE
            EXPB = (127.0 - 0.0564) * 8388608.0
            # 3 tiles per group drain on DVE (ACT alone can't release PSUM
            # buffers fast enough: its 2.05us/tile + accumulator-read tail
            # stalls the 2-buffer ping-pong); group 4 stays light so the DVE
            # queue is clear for the final combine.
            DVE_B_PER_G = {4: (1, 4)}
            DVE_B_DEFAULT = (1, 4, 6)

            def emit_fast_exp(g, b, ps, cw):
                ei = scratch.tile([P, GW], i32, tag="ei", name=f"ei{g}_{b}")
                nc.vector.tensor_scalar(
                    out=ei[:, :cw], in0=ps[:, :cw],
                    scalar1=EXPA, scalar2=EXPB, op0=OP.mult, op1=OP.add,
                )
                nc.vector.tensor_reduce(
                    out=sums[:, b * NG + g: b * NG + g + 1],
                    in_=ei[:, :cw].bitcast(f32),
                    axis=mybir.AxisListType.X, op=OP.add,
                )

            # ---- main GEMM + exp/accumulate ----
            with tc.tile_pool(name="psum", bufs=2, space="PSUM") as psum_pool:
                for g, (c0, cw) in enumerate(groups):
                    nbank = (cw + 511) // 512
                    # per-bank DMAs: the first matmul only waits for 256KB,
                    # not the whole 1MB group
                    wt_g = wtp.tile([P, NK, cw], fp8, tag="wt", name=f"wt{g}")
                    for bank in range(nbank):
                        s0 = bank * 512
                        sw = min(512, cw - s0)
                        nc.sync.dma_start(
                            wt_g[:, :, s0:s0 + sw],
                            wt_d[:, :, c0 + s0:c0 + s0 + sw],
                        )
                        if g == 0 and bank == 0:
                            # second xt half can land after the first bank
                            nc.sync.dma_start(xt_sb[:, 2:4, :], xt_d[:, 2:4, :])
                    if g == 1:
                        # GEMM-critical DMAs for groups 0-1 are queued; the
                        # aux path (slack until ~90us) loads behind them.
                        emit_aux_loads()
                        emit_gathers()
                    for b in range(NB):
                        ps = psum_pool.tile([P, cw], f32, tag="ps", name=f"ps{g}_{b}")
                        for bank in range(nbank):
                            s0 = bank * 512
                            sw = min(512, cw - s0)
                            for kd in range(NKD):
                                nc.tensor.matmul(
                                    ps[:, s0:s0 + sw],
                                    lhsT=xt_sb[:, 2 * kd:2 * kd + 2,
                                               b * P:(b + 1) * P],
                                    rhs=wt_g[:, 2 * kd:2 * kd + 2, s0:s0 + sw],
                                    start=(kd == 0), stop=(kd == NKD - 1),
                                    perf_mode=PM.DoubleRow,
                                )
                        if b in DVE_B_PER_G.get(g, DVE_B_DEFAULT):
                            emit_fast_exp(g, b, ps, cw)
                        else:
                            es = scratch.tile([P, cw], fp8, tag="es",
                                              name=f"es{g}_{b}")
                            nc.scalar.activation(
                                es[:, :cw], ps[:, :cw], AF.Exp,
                                scale=1.0 / WSCALE,
                                accum_out=sums[:, b * NG + g: b * NG + g + 1],
                            )
                    # spread the aux row-blocks across groups 1-4
                    for b_aux in {1: (0, 1), 2: (2, 3), 3: (4, 5),
                                  4: (6, 7)}.get(g, ()):
                        emit_aux_compute(b_aux)
                    if g == 3:
                        # partial class-group sums over groups 0-3: only the
                        # g4 column is left for the tail chain
                        for b in range(NB):
                            nc.vector.tensor_reduce(
                                out=se03[:, b:b + 1],
                                in_=sums[:, b * NG:b * NG + 4],
                                axis=mybir.AxisListType.X, op=OP.add,
                            )
                    if g == 4:
                        # 0.25*qcol - tcol, ready before the tail chain
                        nc.vector.scalar_tensor_tensor(
                            out=pre[:, :], in0=qcol[:, :], scalar=0.25,
                            in1=tcol[:, :], op0=OP.mult, op1=OP.subtract,
                        )

            # ---- logsumexp + per-row combine (minimal serial tail) ----
            nc.vector.tensor_add(
                se[:, :], se03[:, :], sums[:, NG - 1:NB * NG:NG])
            nc.scalar.activation(lse[:, :], se[:, :], AF.Ln)
            nc.vector.tensor_add(ctr2[:, :], lse[:, :], pre[:, :])
            nc.vector.tensor_reduce(
                out=rowtot[:, :], in_=ctr2[:, :],
                axis=mybir.AxisListType.X, op=OP.add,
            )

            # ---- cross-partition sum via ones-matmul, write scalar ----
            with tc.tile_pool(name="psum2", bufs=1, space="PSUM") as pp2:
                tot_ps = pp2.tile([1, 1], f32, name="tot_ps")
                nc.tensor.matmul(
                    tot_ps[:, :], lhsT=rowtot[:, :], rhs=ones[:, :],
                    start=True, stop=True,
                )
                tot_sb = persist.tile([1, 1], f32, name="tot_sb")
                nc.vector.tensor_copy(tot_sb[:, :], tot_ps[:, :])
                nc.sync.dma_start(out_d[:, :], tot_sb[:, :])

    orig_to_json = nc.to_json_bytes
    nc.to_json_bytes = lambda: _patch_bir_bytes(orig_to_json())
    return nc


_NC = None


def _get_nc():
    global _NC
    if _NC is None:
        _NC = _build_bass()
    return _NC


def _make_in_maps(soft_x, hard_x, targets, centers, weight):
    import ml_dtypes

    soft_x = np.asarray(soft_x, np.float32)
    hard_x = np.asarray(hard_x, np.float32)
    targets = np.asarray(targets)
    weight = np.asarray(weight, np.float32)
    centers = np.asarray(centers, np.float32)

    # wt8[p, s, c] = W[c, s*128+p] * WSCALE, fp8e4m3 (TRN flavor: max +-240)
    wt8 = np.ascontiguousarray(
        (weight.T * WSCALE).reshape(NK, P, C).transpose(1, 0, 2)
    ).astype(ml_dtypes.float8_e4m3)
    w_b = np.ascontiguousarray(weight).astype(ml_dtypes.bfloat16)
    cen_b = np.ascontiguousarray(centers).astype(ml_dtypes.bfloat16)

    in_maps = []
    for i in range(N_CORES):
        sl = slice(i * BS, (i + 1) * BS)
        X = np.concatenate([soft_x[sl], hard_x[sl]], axis=0)
        # xt8[p, s, r] = X[r, s*128+p]
        XT8 = np.ascontiguousarray(
            X.T.reshape(NK, P, B).transpose(1, 0, 2)
        ).astype(ml_dtypes.float8_e4m3)
        tg = np.ascontiguousarray(targets[sl].astype(np.int32).reshape(BS, 1))
        in_maps.append(
            {"xt": XT8, "x": X.astype(ml_dtypes.bfloat16), "wt": wt8,
             "w": w_b, "cen": cen_b, "tgt": tg}
        )
    return in_maps


def _run(inputs, trace=False):
    from concourse.bass_utils import run_bass_kernel_spmd

    nc = _get_nc()
    in_maps = _make_in_maps(**inputs)
    res = run_bass_kernel_spmd(
        nc, in_maps, core_ids=list(range(N_CORES)), trace=trace
    )
    total = sum(float(r["out"][0, 0]) for r in res.results)
    return np.float32(total / B_FULL), res


def kernel(soft_x, hard_x, targets, centers, weight):
    loss, _ = _run(
        dict(soft_x=soft_x, hard_x=hard_x, targets=targets,
             centers=centers, weight=weight)
    )
    return loss



# revision 26
# speedup vs baseline: 1.1448x; 1.1448x over previous
"""DPQ joint classification loss on 8 Trainium2 NeuronCores.

reference math (B=4096, D=512, C=10000):
    soft_pred = soft_x @ weight.T ; hard_pred = hard_x @ weight.T
    loss = CE(soft_pred, t) + CE(hard_pred, t)
           + 0.5 * 0.5*(||soft_x - centers[t]||^2 + ||hard_x - centers[t]||^2) / B

Sharding: data-parallel over batch. Core i gets soft rows [i*512,(i+1)*512)
and the matching hard rows, stacked into X = [1024, 512]; weight/centers are
replicated. Each core returns one scalar:
    sum_rows( logsumexp(X @ W^T) - logit_at_target + 0.25*||X - centers[t]||^2 )
and the host computes loss = sum(cores) / B.

Per-core pipeline:
  - PE: fp8e4m3 DoubleRow GEMM (2 fp8 MACs/cell/cycle), rows on partitions
    (8 chunks of 128), classes streamed in 512-wide PSUM banks (4 banks per
    group), fp32 accumulation over 2 double-k chunks of 256.  The weight is
    pre-scaled by 2^12 on the host so its +-0.024 entries land in e4m3's
    normal range; the 2^-12 is folded into the exp activation's free affine
    scale.  The logits only feed logsumexp, which is insensitive to the
    ~1-2% fp8 quantization noise; the target logit is computed exactly in
    fp32 on the DVE side path.
  - ACT: exp straight out of PSUM with fused per-row accumulation
    (no max-subtraction: logits are ~N(0, 0.31), exp is safe in fp32).
  - GPSIMD: indirect-DMA row gathers weight[targets], centers[targets].
  - DVE: exact fp32 target-logit (rowsum(x * w_gather)) and quantization
    (rowsum((x - c_gather)^2)) terms, final per-row combine.
  - PE again: cross-partition sum via ones-matmul; DMA scalar out.
"""

import json

import numpy as np

B_FULL = 4096
D = 512
C = 10000
N_CORES = 8
BS = B_FULL // N_CORES          # 512 rows per core per tensor
B = 2 * BS                      # 1024 stacked rows per core
P = 128
NB = B // P                     # 8 row chunks
NK = D // P                     # 4 contraction chunks
NKD = NK // 2                   # 2 DoubleRow chunks of 256
GW = 2048                       # class-group width = 4 PSUM banks
PARAM = 0.5
WSCALE = 4096.0                 # weight pre-scale: lifts fp8 w out of subnormals


def _patch_bir_bytes(b: bytes, max_waits: int = 1) -> bytes:
    """Adapt Tile-emitted BIR to this walrus build: it supports only one
    sync-wait per instruction (excess waits move to preceding NoOps) and
    rejects the EVENT_SEMAPHORE_RANGE_CLEAR raw-ISA encoding (replaced by
    per-semaphore write-0 EventSemaphore ops)."""
    d = json.loads(b)
    for f in d["functions"]:
        for blk in f["blocks"]:
            new_insts = []
            for ins in blk["instructions"]:
                if (
                    ins.get("opcode") == "ISA"
                    and ins.get("op_name") == "EVENT_SEMAPHORE_RANGE_CLEAR"
                ):
                    # distribute the clears round-robin over all engines:
                    # they sit between the two epilogue barriers, where every
                    # engine is idle, so 5-way parallel beats serial-on-Pool
                    ad = ins.get("ant_dict") or {}
                    engines = ["Pool", "Activation", "PE", "DVE", "SP"]
                    for j, sem_id in enumerate(
                        range(ad["range_first"], ad["range_last"] + 1)
                    ):
                        new_insts.append({
                            "name": f"{ins['name']}_clr{sem_id}",
                            "opcode": "EventSemaphore",
                            "engine": engines[j % len(engines)],
                            "ins": [],
                            "outs": [],
                            "debug": ins.get("debug"),
                            "sync_info": {
                                "on_wait": [],
                                "on_update": [{
                                    "ant_name": f"semclr_{sem_id}",
                                    "id": sem_id,
                                    "sync_type": "semaphore",
                                    "update_mode": "sem-wr-imm",
                                    "update_value": 0,
                                }],
                            },
                        })
                    continue
                si = ins.get("sync_info")
                waits = (si or {}).get("on_wait") or []
                if len(waits) > max_waits:
                    extra, keep = waits[:-max_waits], waits[-max_waits:]
                    idx = 0
                    while extra:
                        chunk, extra = extra[:max_waits], extra[max_waits:]
                        new_insts.append({
                            "name": f"{ins['name']}_w{idx}",
                            "opcode": "NoOp",
                            "engine": ins["engine"],
                            "ins": [],
                            "outs": [],
                            "debug": ins.get("debug"),
                            "sync_info": {"on_wait": chunk, "on_update": []},
                        })
                        idx += 1
                    si["on_wait"] = keep
                new_insts.append(ins)
            blk["instructions"] = new_insts
    return json.dumps(d).encode()


def _build_bass():
    import concourse.bass as bass
    import concourse.tile as tile
    from concourse import mybir

    f32 = mybir.dt.float32
    bf16 = mybir.dt.bfloat16
    fp8 = mybir.dt.float8e4
    i32 = mybir.dt.int32
    AF = mybir.ActivationFunctionType
    OP = mybir.AluOpType
    PM = mybir.MatmulPerfMode

    groups = []
    c0 = 0
    while c0 < C:
        groups.append((c0, min(GW, C - c0)))
        c0 += GW
    NG = len(groups)            # 5: 4 x 2048 + 1 x 1808

    nc = bass.Bass()
    # xt8[p, s, r] = X[r, s*128 + p]  (fp8e4m3)
    xt_d = nc.dram_tensor("xt", [P, NK, B], fp8, kind="ExternalInput")
    # aux path tensors in bf16: halves HBM traffic, precision is ample
    # (the aux terms only need ~1e-3 relative accuracy).
    x_d = nc.dram_tensor("x", [B, D], bf16, kind="ExternalInput")
    # wt8[p, s, c] = W[c, s*128 + p] * WSCALE  (fp8e4m3)
    wt_d = nc.dram_tensor("wt", [P, NK, C], fp8, kind="ExternalInput")
    w_d = nc.dram_tensor("w", [C, D], bf16, kind="ExternalInput")
    cen_d = nc.dram_tensor("cen", [C, D], bf16, kind="ExternalInput")
    tgt_d = nc.dram_tensor("tgt", [BS, 1], i32, kind="ExternalInput")
    out_d = nc.dram_tensor("out", [1, 1], f32, kind="ExternalOutput")

    with tile.TileContext(nc) as tc:
        with (
            tc.tile_pool(name="persist", bufs=1) as persist,
            tc.tile_pool(name="wtp", bufs=2) as wtp,
            tc.tile_pool(name="scratch", bufs=3) as scratch,
        ):
            # ---- resident loads ----
            # xt feeds the matmuls: load it first (split per double-k chunk)
            # so the PE can start as soon as the first half lands.
            xt_sb = persist.tile([P, NK, B], fp8, name="xt8")
            nc.sync.dma_start(xt_sb[:, 0:2, :], xt_d[:, 0:2, :])
            # x/tgt feed only the (small) DVE aux path; issue them on the
            # same sync HWDGE queue *behind* the first two weight groups so
            # they never compete with the GEMM-critical transfers (the aux
            # path has ~80us of slack).  gpsimd SWDGE handles the gathers.
            x_sb = []
            tgt_sb = []
            aux_loads = []

            def emit_aux_loads():
                for c in range(BS // P):
                    t = persist.tile([P, 1], i32, tag=f"tgt{c}", name=f"tgt{c}")
                    nc.sync.dma_start(t[:, :], tgt_d[c * P:(c + 1) * P, :])
                    tgt_sb.append(t)
                for b in range(NB):
                    t = persist.tile([P, D], bf16, tag=f"x{b}", name=f"x{b}")
                    nc.sync.dma_start(t[:, :], x_d[b * P:(b + 1) * P, :])
                    x_sb.append(t)

            # ---- gathers: weight[targets], centers[targets] ----
            wg_sb, cg_sb = [], []

            def emit_gathers():
                for c in range(BS // P):
                    wg = persist.tile([P, D], bf16, tag=f"wg{c}", name=f"wg{c}")
                    nc.gpsimd.indirect_dma_start(
                        out=wg[:, :], out_offset=None, in_=w_d[:, :],
                        in_offset=bass.IndirectOffsetOnAxis(
                            ap=tgt_sb[c][:, :1], axis=0),
                    )
                    wg_sb.append(wg)
                    cg = persist.tile([P, D], bf16, tag=f"cg{c}", name=f"cg{c}")
                    nc.gpsimd.indirect_dma_start(
                        out=cg[:, :], out_offset=None, in_=cen_d[:, :],
                        in_offset=bass.IndirectOffsetOnAxis(
                            ap=tgt_sb[c][:, :1], axis=0),
                    )
                    cg_sb.append(cg)

            # ---- small result tiles ----
            sums = persist.tile([P, NB * NG], f32, name="sums")
            se03 = persist.tile([P, NB], f32, name="se03")
            se = persist.tile([P, NB], f32, name="se")
            lse = persist.tile([P, NB], f32, name="lse")
            tcol = persist.tile([P, NB], f32, name="tcol")
            qcol = persist.tile([P, NB], f32, name="qcol")
            pre = persist.tile([P, NB], f32, name="pre")
            ctr2 = persist.tile([P, NB], f32, name="ctr2")
            rowtot = persist.tile([P, 1], f32, name="rowtot")
            ones = persist.tile([P, 1], f32, name="ones")
            nc.vector.memset(ones[:, :], 1.0)
            junk = persist.tile([P, 512], bf16, name="junk")
            nc.vector.memset(junk[:, :], 0.5)

            # ---- PE pre-warm ----
            # throwaway matmuls during the head DMA wait flip the HAM clock
            # gate to 8/8 (needs ~3.4us of sustained PE activity), so the
            # first real matmuls run at 2.4GHz instead of 1.2GHz.
            with tc.tile_pool(name="warm", bufs=1, space="PSUM") as warm_pool:
                wps = warm_pool.tile([P, 512], f32, name="wps")
                for i in range(7):
                    nc.tensor.matmul(
                        wps[:, :], lhsT=junk[:, :P], rhs=junk[:, :],
                        start=True, stop=True,
                    )

            # ---- aux path on DVE: target logits + quantization ----
            # (tensor_tensor_reduce lowers to a raw DVE ISA encoding this
            # walrus rejects, so use separate mul/sub + reduce ops; bf16
            # intermediates get the DVE 2x modes).  Emitted per row-block,
            # interleaved into the main loop so the DVE queue services them
            # between fast-exp tiles instead of serially at the end.
            def emit_aux_compute(b):
                c = b % (BS // P)
                pr = scratch.tile([P, D], bf16, tag="pr", name=f"pr{b}")
                nc.vector.tensor_mul(pr[:, :], x_sb[b][:, :], wg_sb[c][:, :])
                nc.vector.tensor_reduce(
                    out=tcol[:, b:b + 1], in_=pr[:, :],
                    axis=mybir.AxisListType.X, op=OP.add,
                )
                df = scratch.tile([P, D], bf16, tag="df", name=f"df{b}")
                nc.vector.tensor_sub(df[:, :], x_sb[b][:, :], cg_sb[c][:, :])
                sq = scratch.tile([P, D], bf16, tag="sq", name=f"sq{b}")
                nc.vector.tensor_mul(sq[:, :], df[:, :], df[:, :])
                nc.vector.tensor_reduce(
                    out=qcol[:, b:b + 1], in_=sq[:, :],
                    axis=mybir.AxisListType.X, op=OP.add,
                )

            # Fast exp on DVE for 2 of 8 row-blocks per group (ScalarE is
            # otherwise the bottleneck at ~82us vs PE ~78us).  Schraudolph:
            # bits(2^t) ~= A*t + B with the sawtooth mean-corrected via B;
            # psum is drained to an i32 SBUF tile first so the PSUM buffer
            # is freed at ACT-like pace, then reduced via a free bitcast.
            EXPA = (2.0 ** 23) * 1.4426950408889634 / WSCALE
            EXPB = (127.0 - 0.0564) * 8388608.0
            # 2 tiles per group drain on DVE (3 overloads it: each fast-exp
            # costs ~4.6us of 0.96GHz DVE); group 0 takes 3 since the aux
            # inputs aren't loaded yet and ACT starts ~3.5us behind the PE;
            # group 4 stays light for the final combine.
            DVE_B_PER_G = {0: (1, 3, 5), 4: (2, 5)}
            DVE_B_DEFAULT = (3, 6)

            def emit_fast_exp(g, b, ps, cw):
                ei = scratch.tile([P, GW], i32, tag="ei", name=f"ei{g}_{b}")
                nc.vector.tensor_scalar(
                    out=ei[:, :cw], in0=ps[:, :cw],
                    scalar1=EXPA, scalar2=EXPB, op0=OP.mult, op1=OP.add,
                )
                nc.vector.tensor_reduce(
                    out=sums[:, b * NG + g: b * NG + g + 1],
                    in_=ei[:, :cw].bitcast(f32),
                    axis=mybir.AxisListType.X, op=OP.add,
                )

            # ---- main GEMM + exp/accumulate ----
            with tc.tile_pool(name="psum", bufs=2, space="PSUM") as psum_pool:
                for g, (c0, cw) in enumerate(groups):
                    nbank = (cw + 511) // 512
                    # per-bank DMAs: the first matmul only waits for 256KB,
                    # not the whole 1MB group
                    wt_g = wtp.tile([P, NK, cw], fp8, tag="wt", name=f"wt{g}")
                    for bank in range(nbank):
                        s0 = bank * 512
                        sw = min(512, cw - s0)
                        nc.sync.dma_start(
                            wt_g[:, :, s0:s0 + sw],
                            wt_d[:, :, c0 + s0:c0 + s0 + sw],
                        )
                        if g == 0 and bank == 0:
                            # second xt half can land after the first bank
                            nc.sync.dma_start(xt_sb[:, 2:4, :], xt_d[:, 2:4, :])
                    if g == 1:
                        # GEMM-critical DMAs for groups 0-1 are queued; the
                        # aux path (slack until ~90us) loads behind them.
                        emit_aux_loads()
                        emit_gathers()
                    for b in range(NB):
                        ps = psum_pool.tile([P, cw], f32, tag="ps", name=f"ps{g}_{b}")
                        for bank in range(nbank):
                            s0 = bank * 512
                            sw = min(512, cw - s0)
                            for kd in range(NKD):
                                nc.tensor.matmul(
                                    ps[:, s0:s0 + sw],
                                    lhsT=xt_sb[:, 2 * kd:2 * kd + 2,
                                               b * P:(b + 1) * P],
                                    rhs=wt_g[:, 2 * kd:2 * kd + 2, s0:s0 + sw],
                                    start=(kd == 0), stop=(kd == NKD - 1),
                                    perf_mode=PM.DoubleRow,
                                )
                        if b in DVE_B_PER_G.get(g, DVE_B_DEFAULT):
                            emit_fast_exp(g, b, ps, cw)
                        else:
                            es = scratch.tile([P, cw], fp8, tag="es",
                                              name=f"es{g}_{b}")
                            nc.scalar.activation(
                                es[:, :cw], ps[:, :cw], AF.Exp,
                                scale=1.0 / WSCALE,
                                accum_out=sums[:, b * NG + g: b * NG + g + 1],
                            )
                    # spread the aux row-blocks across groups 1-4
                    for b_aux in {1: (0, 1), 2: (2, 3), 3: (4, 5),
                                  4: (6, 7)}.get(g, ()):
                        emit_aux_compute(b_aux)
                    if g == 3:
                        # partial class-group sums over groups 0-3: only the
                        # g4 column is left for the tail chain
                        for b in range(NB):
                            nc.vector.tensor_reduce(
                                out=se03[:, b:b + 1],
                                in_=sums[:, b * NG:b * NG + 4],
                                axis=mybir.AxisListType.X, op=OP.add,
                            )
                    if g == 4:
                        # 0.25*qcol - tcol, ready before the tail chain
                        nc.vector.scalar_tensor_tensor(
                            out=pre[:, :], in0=qcol[:, :], scalar=0.25,
                            in1=tcol[:, :], op0=OP.mult, op1=OP.subtract,
                        )

            # ---- logsumexp + per-row combine (minimal serial tail) ----
            nc.vector.tensor_add(
                se[:, :], se03[:, :], sums[:, NG - 1:NB * NG:NG])
            nc.scalar.activation(lse[:, :], se[:, :], AF.Ln)
            nc.vector.tensor_add(ctr2[:, :], lse[:, :], pre[:, :])
            nc.vector.tensor_reduce(
                out=rowtot[:, :], in_=ctr2[:, :],
                axis=mybir.AxisListType.X, op=OP.add,
            )

            # ---- cross-partition sum via ones-matmul, write scalar ----
            with tc.tile_pool(name="psum2", bufs=1, space="PSUM") as pp2:
                tot_ps = pp2.tile([1, 1], f32, name="tot_ps")
                nc.tensor.matmul(
                    tot_ps[:, :], lhsT=rowtot[:, :], rhs=ones[:, :],
                    start=True, stop=True,
                )
                tot_sb = persist.tile([1, 1], f32, name="tot_sb")
                nc.vector.tensor_copy(tot_sb[:, :], tot_ps[:, :])
                nc.sync.dma_start(out_d[:, :], tot_sb[:, :])

    orig_to_json = nc.to_json_bytes
    nc.to_json_bytes = lambda: _patch_bir_bytes(orig_to_json())
    return nc


_NC = None


def _get_nc():
    global _NC
    if _NC is None:
        _NC = _build_bass()
    return _NC


def _make_in_maps(soft_x, hard_x, targets, centers, weight):
    import ml_dtypes

    soft_x = np.asarray(soft_x, np.float32)
    hard_x = np.asarray(hard_x, np.float32)
    targets = np.asarray(targets)
    weight = np.asarray(weight, np.float32)
    centers = np.asarray(centers, np.float32)

    # wt8[p, s, c] = W[c, s*128+p] * WSCALE, fp8e4m3 (TRN flavor: max +-240)
    wt8 = np.ascontiguousarray(
        (weight.T * WSCALE).reshape(NK, P, C).transpose(1, 0, 2)
    ).astype(ml_dtypes.float8_e4m3)
    w_b = np.ascontiguousarray(weight).astype(ml_dtypes.bfloat16)
    cen_b = np.ascontiguousarray(centers).astype(ml_dtypes.bfloat16)

    in_maps = []
    for i in range(N_CORES):
        sl = slice(i * BS, (i + 1) * BS)
        X = np.concatenate([soft_x[sl], hard_x[sl]], axis=0)
        # xt8[p, s, r] = X[r, s*128+p]
        XT8 = np.ascontiguousarray(
            X.T.reshape(NK, P, B).transpose(1, 0, 2)
        ).astype(ml_dtypes.float8_e4m3)
        tg = np.ascontiguousarray(targets[sl].astype(np.int32).reshape(BS, 1))
        in_maps.append(
            {"xt": XT8, "x": X.astype(ml_dtypes.bfloat16), "wt": wt8,
             "w": w_b, "cen": cen_b, "tgt": tg}
        )
    return in_maps


def _run(inputs, trace=False):
    from concourse.bass_utils import run_bass_kernel_spmd

    nc = _get_nc()
    in_maps = _make_in_maps(**inputs)
    res = run_bass_kernel_spmd(
        nc, in_maps, core_ids=list(range(N_CORES)), trace=trace
    )
    total = sum(float(r["out"][0, 0]) for r in res.results)
    return np.float32(total / B_FULL), res


def kernel(soft_x, hard_x, targets, centers, weight):
    loss, _ = _run(
        dict(soft_x=soft_x, hard_x=hard_x, targets=targets,
             centers=centers, weight=weight)
    )
    return loss

